# revision 1
# baseline (speedup 1.0000x reference)
"""GAT+JumpingKnowledge GNN kernel for 8 Trainium2 NeuronCores.

Sharding: nodes are partitioned across 8 cores by dst ownership (6250/core).
Each core, per layer:
  - projects its own nodes' features h = x @ [W | W@a_src | W@a_dst]
  - writes them as packed 256B table rows [64 x fp16 h | f32 alpha_src | f32
    alpha_dst | pad]
  - AllGathers the table (full 50176-row table on every core)
  - gathers, per dst-node "slot grid" (nodes on partitions, incoming-edge
    rounds on the free dim), the src rows of its edges via dma_gather
    (int16 indices -> the table is addressed as two 25088-row halves)
  - computes the edge softmax (no max subtraction; logit range is ~[-7, 7])
    and the weighted aggregation with DVE multiply + free-dim reduce
Final JK-max + output projection happen on the owned nodes; the host
reassembles and un-permutes the full [50000, 40] output.
"""

import numpy as np

# --- problem constants (hardcoded per harness contract) ---
N = 50000
E = 1600000
F_IN = 128
H = 64
L = 3
OUT = 40
NEG_SLOPE = 0.2
NC = 8
NPC_REAL = N // NC          # 6250 real nodes per core
BLOCKS = 49                 # ceil(6250/128)
NPC = BLOCKS * 128          # 6272 padded nodes per core
TAB_ROWS = NC * NPC         # 50176
TAB_HALF = TAB_ROWS // 2    # 25088 (= rows of cores 0..3)
SB_BLOCKS = 1               # blocks per superblock (gather granularity)
DUMMY_LOCAL = NPC_REAL      # local row 6250 is a pad row on every core
ALPHA_NEG = -1.0e30


# ---------------------------------------------------------------------------
# Host-side graph preprocessing
# ---------------------------------------------------------------------------

def _fill_grid(Rn, slot_p, rows_vals, dummy):
    """Grid [Rn, 128] in i=r*128+p order; node p's edges fill rounds 0..k-1."""
    grid = np.full((int(Rn), 128), dummy, np.int64)
    o = np.argsort(slot_p, kind="stable")
    ps = slot_p[o]
    rv = rows_vals[o]
    first = np.searchsorted(ps, np.arange(128), side="left")
    ranks = np.arange(len(ps)) - first[ps]
    grid[ranks, ps] = rv
    return grid.reshape(-1)


def _preprocess(edge_index):
    src = np.concatenate([edge_index[0], np.arange(N, dtype=np.int64)]).astype(np.int64)
    dst = np.concatenate([edge_index[1], np.arange(N, dtype=np.int64)]).astype(np.int64)
    is_lo = (src // NPC_REAL) < (NC // 2)   # table half A iff src owned by cores 0-3

    perms = []
    inv_perms = np.zeros((NC, NPC_REAL), np.int64)
    edges_by_core = []
    for c in range(NC):
        lo, hi = c * NPC_REAL, (c + 1) * NPC_REAL
        m = (dst >= lo) & (dst < hi)
        s_c = src[m]
        d_c = dst[m] - lo
        k_lo = np.bincount(d_c[is_lo[m]], minlength=NPC_REAL)
        k_hi = np.bincount(d_c[~is_lo[m]], minlength=NPC_REAL)
        order = np.lexsort((-(k_lo + k_hi), -np.maximum(k_lo, k_hi)))
        perm = np.full(NPC, -1, np.int64)
        perm[:NPC_REAL] = order
        inv_perms[c, order] = np.arange(NPC_REAL)
        perms.append(perm)
        edges_by_core.append((s_c, d_c, k_lo, k_hi))

    def table_row(gids):
        c = gids // NPC_REAL
        return c * NPC + inv_perms[c, gids - c * NPC_REAL]

    # shared (cross-core max) per-block round counts
    RL = np.zeros(BLOCKS, np.int64)
    RH = np.zeros(BLOCKS, np.int64)
    for c in range(NC):
        _, _, k_lo, k_hi = edges_by_core[c]
        perm = perms[c]
        for bidx in range(BLOCKS):
            nodes = perm[bidx * 128:(bidx + 1) * 128]
            nodes = nodes[nodes >= 0]
            if len(nodes):
                RL[bidx] = max(RL[bidx], k_lo[nodes].max())
                RH[bidx] = max(RH[bidx], k_hi[nodes].max())

    idx_a_cores, idx_b_cores = [], []
    for c in range(NC):
        s_c, d_c, _, _ = edges_by_core[c]
        slot_of = inv_perms[c, d_c]
        rows = table_row(s_c)
        lo_m = rows < TAB_HALF
        la, lb = [], []
        for bidx in range(BLOCKS):
            base = bidx * 128
            in_blk = (slot_of >= base) & (slot_of < base + 128)
            sel = in_blk & lo_m
            la.append(_fill_grid(RL[bidx], slot_of[sel] - base, rows[sel],
                                 DUMMY_LOCAL))
            sel = in_blk & ~lo_m
            lb.append(_fill_grid(RH[bidx], slot_of[sel] - base,
                                 rows[sel] - TAB_HALF, DUMMY_LOCAL))
        idx_a_cores.append(np.concatenate(la).astype(np.int16))
        idx_b_cores.append(np.concatenate(lb).astype(np.int16))

    return perms, idx_a_cores, idx_b_cores, RL, RH


def _pad_neg_col():
    col = np.zeros((128, 1), np.float32)
    col[NPC_REAL - (BLOCKS - 1) * 128:] = ALPHA_NEG
    return col


def _wrap_idx(flat):
    """[num] -> [128, num//16] wrapped (i%16, i//16), replicated to 128 parts."""
    num = len(flat)
    assert num % 16 == 0
    w = flat.reshape(num // 16, 16).T
    return np.ascontiguousarray(np.tile(w, (8, 1))).astype(np.int16)


# ---------------------------------------------------------------------------
# Device kernel builder
# ---------------------------------------------------------------------------

def _build(nc, RL, RH, n_idx_a, n_idx_b, stage="full"):
    import contextlib

    import concourse.mybir as mybir
    import concourse.tile as tile
    from concourse import library_config
    from concourse.masks import make_identity

    f32 = mybir.dt.float32
    f16 = mybir.dt.float16
    AF = mybir.ActivationFunctionType
    ALU = mybir.AluOpType

    # --- I/O ---
    x_in = nc.dram_tensor("x_own", [NPC, F_IN], f32, kind="ExternalInput").ap()
    w1_in = nc.dram_tensor("w1", [F_IN, H], f32, kind="ExternalInput").ap()
    w23_in = nc.dram_tensor("w23", [L - 1, H, H], f32, kind="ExternalInput").ap()
    asrc_in = nc.dram_tensor("asrc", [L, H], f32, kind="ExternalInput").ap()
    adst_in = nc.dram_tensor("adst", [L, H], f32, kind="ExternalInput").ap()
    bias_in = nc.dram_tensor("bias", [L, H], f32, kind="ExternalInput").ap()
    wout_in = nc.dram_tensor("wout", [H, OUT], f32, kind="ExternalInput").ap()
    bout_in = nc.dram_tensor("bout", [1, OUT], f32, kind="ExternalInput").ap()
    idxa_in = nc.dram_tensor("idx_a", [128, n_idx_a // 16], mybir.dt.int16,
                             kind="ExternalInput").ap()
    idxb_in = nc.dram_tensor("idx_b", [128, n_idx_b // 16], mybir.dt.int16,
                             kind="ExternalInput").ap()
    padneg_in = nc.dram_tensor("pad_neg", [128, 1], f32, kind="ExternalInput").ap()
    out_t = nc.dram_tensor("y", [NPC, OUT], f32, kind="ExternalOutput").ap()

    # --- internal DRAM ---
    tab_own = nc.dram_tensor("tab_own", [NPC, H], f32, kind="Internal").ap()
    tab_full = nc.dram_tensor("tab_full", [TAB_ROWS, H], f32, kind="Internal",
                              addr_space="Shared").ap()

    R_TOT = [int(RL[b] + RH[b]) for b in range(BLOCKS)]
    R_MAX = max(R_TOT)

    sbs = []
    b0 = 0
    while b0 < BLOCKS:
        sbs.append(list(range(b0, min(b0 + SB_BLOCKS, BLOCKS))))
        b0 += SB_BLOCKS

    with tile.TileContext(nc) as tc:
        nc.gpsimd.load_library(library_config.mlp)

        with contextlib.ExitStack() as ctx:
            const = ctx.enter_context(tc.tile_pool(name="const", bufs=1))
            psum = ctx.enter_context(tc.tile_pool(name="psum", bufs=2, space="PSUM"))
            sb_pool = ctx.enter_context(tc.tile_pool(name="grids", bufs=4))
            work = ctx.enter_context(tc.tile_pool(name="work", bufs=2))
            small = ctx.enter_context(tc.tile_pool(name="small", bufs=3))

            ident = const.tile([128, 128], f32, tag="ident")
            make_identity(nc, ident[:])
            ones_row = const.tile([1, 128], f32, tag="ones")
            nc.vector.memset(ones_row[:], 1.0)
            idxa_sb = const.tile([128, n_idx_a // 16], mybir.dt.int16, tag="idxa")
            nc.sync.dma_start(idxa_sb[:], idxa_in[:])
            idxb_sb = const.tile([128, n_idx_b // 16], mybir.dt.int16, tag="idxb")
            nc.sync.dma_start(idxb_sb[:], idxb_in[:])
            x_buf = const.tile([128, BLOCKS * F_IN], f32, tag="xbuf")
            nc.sync.dma_start(
                x_buf[:].rearrange("p (t f) -> p t f", t=BLOCKS),
                x_in.rearrange("(t p) f -> p t f", p=128),
            )
            jk_buf = const.tile([128, BLOCKS * H], f32, tag="jkbuf")
            alphad = const.tile([128, BLOCKS], f32, tag="alphad")
            pad_neg = const.tile([128, 1], f32, tag="padneg")
            nc.sync.dma_start(pad_neg[:], padneg_in[:])

            self_q = [0]
            for layer in range(L if stage == "full" else 1):
                F = F_IN if layer == 0 else H
                w_ap = w1_in if layer == 0 else w23_in[layer - 1]

                # Waug = [W | W@a_src | W@a_dst]  [F, H+2]
                waug = small.tile([128, H + 2], f32, tag="waug")
                nc.sync.dma_start(waug[:F, 0:H], w_ap)
                wt_ps = psum.tile([H, 128], f32, tag="ps_t")
                nc.tensor.transpose(wt_ps[:, :F], waug[:F, 0:H], ident[:F, :F])
                wt_sb = small.tile([H, 128], f32, tag="wtsb")
                nc.scalar.copy(wt_sb[:, :F], wt_ps[:, :F])
                a_cols = small.tile([H, 2], f32, tag="acols")
                nc.sync.dma_start(a_cols[:, 0:1], asrc_in[layer, :, None])
                nc.sync.dma_start(a_cols[:, 1:2], adst_in[layer, :, None])
                va_ps = psum.tile([128, 2], f32, tag="ps_m")
                nc.tensor.matmul(va_ps[:F, :], wt_sb[:, :F], a_cols[:],
                                 start=True, stop=True)
                nc.vector.tensor_copy(waug[:F, H:H + 2], va_ps[:F, :])

                # bias row -> [128, H] broadcast tile
                b_row = small.tile([1, H], f32, tag="brow")
                nc.sync.dma_start(b_row[:], bias_in[layer, None, :])
                bt_ps = psum.tile([128, H], f32, tag="ps_m")
                nc.tensor.matmul(bt_ps[:], ones_row[:], b_row[:],
                                 start=True, stop=True)
                b_tile = small.tile([128, H], f32, tag="btile")
                nc.scalar.copy(b_tile[:], bt_ps[:])

                # project own nodes, pack + store table rows
                for t in range(BLOCKS):
                    xt = x_buf[:, t * F:(t + 1) * F]
                    xT_ps = psum.tile([F, 128], f32, tag="ps_t")
                    nc.tensor.transpose(xT_ps[:], xt, ident[:])
                    xT_sb = work.tile([F, 128], f32, tag="xTsb")
                    nc.scalar.copy(xT_sb[:], xT_ps[:])
                    h_ps = psum.tile([128, H + 2], f32, tag="ps_m")
                    nc.tensor.matmul(h_ps[:], xT_sb[:], waug[:F, :],
                                     start=True, stop=True)
                    row = work.tile([128, H], f32, tag="row")
                    row16 = row[:].bitcast(f16)
                    nc.vector.tensor_copy(row16[:, 0:H], h_ps[:, 0:H])
                    if t == BLOCKS - 1:
                        # pad rows (incl. the dummy target row): alpha_src -> -1e30
                        nc.vector.tensor_tensor(out=row[:, 32:33],
                                                in0=h_ps[:, H:H + 1],
                                                in1=pad_neg[:], op=ALU.add)
                        nc.vector.tensor_copy(row[:, 33:34], h_ps[:, H + 1:H + 2])
                    else:
                        nc.vector.tensor_copy(row[:, 32:34], h_ps[:, H:H + 2])
                    nc.vector.tensor_copy(alphad[:, t:t + 1], h_ps[:, H + 1:H + 2])
                    nc.sync.dma_start(tab_own[t * 128:(t + 1) * 128, :], row[:])

                if stage == "proj":
                    continue
                nc.gpsimd.collective_compute(
                    "AllGather",
                    ALU.bypass,
                    replica_groups=[list(range(NC))],
                    ins=[tab_own.opt()],
                    outs=[tab_full.opt()],
                )
                if stage == "ag":
                    continue

                # edge processing
                off_a = 0
                off_b = 0
                for sb in sbs:
                    na = int(sum(128 * RL[bb] for bb in sb))
                    nb = int(sum(128 * RH[bb] for bb in sb))
                    ga = sb_pool.tile([128, max(na // 128, 1) * H], f32, tag="gridA")
                    gb = sb_pool.tile([128, max(nb // 128, 1) * H], f32, tag="gridB")
                    # dma_gather is capped at 1024 indices per call (SWDGE
                    # descriptor ring); split and round-robin the queues.
                    for grid, n_tot, off, isb, base in (
                        (ga, na, off_a, idxa_sb, tab_full[0:TAB_HALF, :]),
                        (gb, nb, off_b, idxb_sb, tab_full[TAB_HALF:TAB_ROWS, :]),
                    ):
                        done = 0
                        while done < n_tot:
                            step = min(1024, n_tot - done)
                            nc.gpsimd.dma_gather(
                                grid[:].rearrange("p (r h) -> p r h", h=H)
                                [:, done // 128:(done + step) // 128, :],
                                base,
                                isb[:, (off + done) // 16:(off + done + step) // 16],
                                step, step, H,
                                queue_num=self_q[0] % 4,
                            )
                            self_q[0] += 1
                            done += step
                    off_a += na
                    off_b += nb
                    if stage == "gather":
                        continue

                    ra = 0
                    rb = 0
                    for b in sb:
                        rl, rh, rt = int(RL[b]), int(RH[b]), R_TOT[b]
                        ga3 = ga[:].rearrange("p (r h) -> p r h", h=H)
                        gb3 = gb[:].rearrange("p (r h) -> p r h", h=H)
                        tbuf = work.tile([128, R_MAX], f32, tag="tbuf")
                        if rl:
                            nc.scalar.activation(
                                tbuf[:, 0:rl], ga3[:, ra:ra + rl, 32], AF.Identity,
                                bias=alphad[:, b:b + 1])
                        if rh:
                            nc.scalar.activation(
                                tbuf[:, rl:rt], gb3[:, rb:rb + rh, 32], AF.Identity,
                                bias=alphad[:, b:b + 1])
                        # leaky relu: e = max(t, 0.2 t)
                        nc.vector.scalar_tensor_tensor(
                            out=tbuf[:, 0:rt], in0=tbuf[:, 0:rt],
                            scalar=NEG_SLOPE, in1=tbuf[:, 0:rt],
                            op0=ALU.mult, op1=ALU.max)
                        p_t = work.tile([128, R_MAX], f32, tag="ptile")
                        den = small.tile([128, 1], f32, tag="den")
                        nc.scalar.activation(p_t[:, 0:rt], tbuf[:, 0:rt], AF.Exp,
                                             accum_out=den[:])
                        wt = work.tile([128, H * R_MAX], f32, tag="wtile")
                        wt3 = wt[:].rearrange("p (f r) -> p f r", r=R_MAX)
                        if rl:
                            hA = (ga[:].bitcast(f16)
                                  .rearrange("p (r h) -> p r h", h=2 * H)
                                  [:, ra:ra + rl, 0:H])
                            nc.vector.tensor_tensor(
                                out=wt3[:, :, 0:rl].transpose([0, 2, 1]),
                                in0=hA,
                                in1=p_t[:, 0:rl].unsqueeze(2).to_broadcast(
                                    [128, rl, H]),
                                op=ALU.mult)
                        if rh:
                            hB = (gb[:].bitcast(f16)
                                  .rearrange("p (r h) -> p r h", h=2 * H)
                                  [:, rb:rb + rh, 0:H])
                            nc.vector.tensor_tensor(
                                out=wt3[:, :, rl:rt].transpose([0, 2, 1]),
                                in0=hB,
                                in1=p_t[:, rl:rt].unsqueeze(2).to_broadcast(
                                    [128, rh, H]),
                                op=ALU.mult)
                        num = work.tile([128, H], f32, tag="num")
                        nc.vector.reduce_sum(num[:], wt3[:, :, 0:rt],
                                             axis=mybir.AxisListType.X)
                        nc.vector.tensor_scalar_max(den[:], den[:], 1e-30)
                        recip = small.tile([128, 1], f32, tag="recip")
                        nc.vector.reciprocal(recip[:], den[:])
                        if layer < L - 1:
                            xn = x_buf[:, b * H:(b + 1) * H]
                        else:
                            xn = work.tile([128, H], f32, tag="xnlast",
                                           name="xnlast")[:]
                        nc.vector.scalar_tensor_tensor(
                            out=xn, in0=num[:], scalar=recip[:, 0:1],
                            in1=b_tile[:], op0=ALU.mult, op1=ALU.add)
                        nc.vector.tensor_scalar_max(xn, xn, 0.0)
                        jk = jk_buf[:, b * H:(b + 1) * H]
                        if layer == 0:
                            nc.vector.tensor_copy(jk, xn)
                        else:
                            nc.vector.tensor_tensor(out=jk, in0=jk, in1=xn,
                                                    op=ALU.max)
                        ra += rl
                        rb += rh

            # final projection: y = jk @ Wout + bout
            if stage != "full":
                return nc
            wout_sb = const.tile([H, OUT], f32, tag="wout")
            nc.sync.dma_start(wout_sb[:], wout_in[:])
            bo_row = const.tile([1, OUT], f32, tag="borow")
            nc.sync.dma_start(bo_row[:], bout_in[:])
            bo_ps = psum.tile([128, OUT], f32, tag="ps_m")
            nc.tensor.matmul(bo_ps[:], ones_row[:], bo_row[:], start=True, stop=True)
            bo_tile = const.tile([128, OUT], f32, tag="botile")
            nc.scalar.copy(bo_tile[:], bo_ps[:])
            for t in range(BLOCKS):
                jt = jk_buf[:, t * H:(t + 1) * H]
                jT_ps = psum.tile([H, 128], f32, tag="ps_t")
                nc.tensor.transpose(jT_ps[:], jt, ident[:])
                jT_sb = work.tile([H, 128], f32, tag="jTsb")
                nc.scalar.copy(jT_sb[:], jT_ps[:])
                y_ps = psum.tile([128, OUT], f32, tag="ps_m")
                nc.tensor.matmul(y_ps[:], jT_sb[:], wout_sb[:], start=True, stop=True)
                y_sb = work.tile([128, OUT], f32, tag="ysb")
                nc.vector.tensor_tensor(out=y_sb[:], in0=y_ps[:], in1=bo_tile[:],
                                        op=ALU.add)
                nc.sync.dma_start(out_t[t * 128:(t + 1) * 128, :], y_sb[:])

    return nc


# ---------------------------------------------------------------------------
# Entry point
# ---------------------------------------------------------------------------

def kernel(x, edge_index, W1, W23, a_src, a_dst, b, Wout, bout):
    import concourse.bacc as bacc
    from concourse import bass_utils

    x = np.asarray(x, np.float32)
    edge_index = np.asarray(edge_index)
    perms, idx_a, idx_b, RL, RH = _preprocess(edge_index.astype(np.int64))

    n_idx_a = len(idx_a[0])
    n_idx_b = len(idx_b[0])

    nc = bacc.Bacc("TRN2", target_bir_lowering=False, debug=False, num_devices=NC,
                   num_swdge_queues=4)
    _build(nc, RL, RH, n_idx_a, n_idx_b)
    nc.compile()

    in_maps = []
    for c in range(NC):
        perm = perms[c]
        x_own = np.zeros((NPC, F_IN), np.float32)
        valid = np.nonzero(perm >= 0)[0]
        x_own[valid] = x[c * NPC_REAL + perm[valid]]
        in_maps.append({
            "x_own": x_own,
            "w1": np.asarray(W1, np.float32),
            "w23": np.asarray(W23, np.float32),
            "asrc": np.asarray(a_src, np.float32),
            "adst": np.asarray(a_dst, np.float32),
            "bias": np.asarray(b, np.float32),
            "wout": np.asarray(Wout, np.float32),
            "bout": np.asarray(bout, np.float32).reshape(1, OUT),
            "idx_a": _wrap_idx(idx_a[c]),
            "idx_b": _wrap_idx(idx_b[c]),
            "pad_neg": _pad_neg_col(),
        })

    res = bass_utils.run_bass_kernel_spmd(nc, in_maps, core_ids=list(range(NC)))
    global _last_results
    _last_results = res
    out = np.zeros((N, OUT), np.float32)
    for c in range(NC):
        y = res.results[c]["y"]
        perm = perms[c]
        valid = np.nonzero(perm >= 0)[0]
        out[c * NPC_REAL + perm[valid]] = y[valid]
    return out

